# revision 4
# baseline (speedup 1.0000x reference)
"""Causal self-attention kernel for Trainium2, 8 NeuronCores.

Problem: B=4, T=2048, C=1024, 16 heads, head_dim=64, fp32 in/out.
  q = x@Wq.T, k = x@Wk.T, v = x@Wv.T  (heads split)
  attn = softmax(causal(q@k.T/8)); out = (attn@v) @ Wo.T

Sharding: 8 cores = 4 batches x 2 head-groups (8 heads each).
Each core computes QKV projections for its (batch, head-group),
causal attention, and a partial output projection against its
512 columns of W_o; a pairwise ReduceScatter sums the two
head-group partials and leaves each core with half the T rows.
The host reassembles the full [4, 2048, 1024] output and feeds
pre-transposed x / weight shards so no on-chip transposes are needed.

All matmul operands are bf16 (PSUM accumulation fp32, rel err ~4e-3,
gate is 2e-2). bf16 matches fp32r stream throughput on the PE but
halves DMA/SBUF traffic, enables the compiler's fast-weight-load path
(fp32r weights are excluded from FWL), lifts the fp32r short-moving-dim
penalty so diagonal score tiles trim to the exact causal boundary, and
doubles DVE copy throughput. Softmax skips max-subtraction (scores are
~N(0,1); exp is safe in fp32) and gets the denominator for free by
augmenting v with a ones column (row 64 of the av matmul output); the
ones column is persistent and written only on the first iteration.

Per-core layouts:
  xT   [C, T] streamed per 512-col chunk as [128, 8, 512]
  qT/kT [128, 4, T]         (head pair 2m,2m+1 at partitions 0:64/64:128
                             of plane m)
  v    [128, 16, 8, 65]     (k-tile, head, 64 dims + ones col)
  scores sT [k-tile 128, q 512] per (head, k-tile, q-chunk); both heads
        of a pair run concurrently via PE row-tiling (K=64 each) into
        one 2-bank PSUM tile, one exp covers both
  avT  [65, q 512] accumulated in PSUM over k-tiles; row 64 = denom

Across repeat iterations (used for amortized timing), the next
iteration's chunk-0 q-projection is hoisted into the final attention
chunk's filler slot; its k/v projection follows once the write-after-
read hazard on kT/vt clears.
"""

import ml_dtypes
import numpy as np
from contextlib import ExitStack

import concourse.bass as bass
import concourse.tile as tile
from concourse import bacc, mybir, bass_utils

B, T, C = 4, 2048, 1024
NCORES = 8
NH = 8            # heads per core
HD = 64
S = NH * HD       # 512 = per-core qkv dim shard
TT = T // 128     # 16 T-tiles
CCH = C // 128    # 8 C-chunks
QC = T // 512     # 4 q-chunks of 512
F32 = mybir.dt.float32
F32R = mybir.dt.float32r
BF16 = mybir.dt.bfloat16
EXP = mybir.ActivationFunctionType.Exp
MULT = mybir.AluOpType.mult
RG = [[0, 1], [2, 3], [4, 5], [6, 7]]

_cache = {}


def _build_kernel(collective=True, repeat=1):
    nc = bacc.Bacc("TRN2", target_bir_lowering=False, debug=False,
                   num_devices=NCORES)
    # all matmul operands arrive pre-transposed from the host, fp32r
    xT_d = nc.dram_tensor("xT", [C, T], BF16, kind="ExternalInput").ap()
    wqT_d = nc.dram_tensor("wqT", [C, S], BF16, kind="ExternalInput").ap()
    wkT_d = nc.dram_tensor("wkT", [C, S], BF16, kind="ExternalInput").ap()
    wvT_d = nc.dram_tensor("wvT", [C, S], BF16, kind="ExternalInput").ap()
    woT_d = nc.dram_tensor("woT", [S, C], BF16, kind="ExternalInput").ap()
    oh_d = nc.dram_tensor("o_half", [T // 2, C], F32,
                          kind="ExternalOutput").ap()

    with tile.TileContext(nc) as tc, ExitStack() as top:
        const = top.enter_context(tc.tile_pool(name="const", bufs=1))
        dram = top.enter_context(tc.tile_pool(name="dram", bufs=1,
                                              space="DRAM"))
        # tri[kk, u] = 1 if u >= kk else 0 (keep where q >= k on the diag)
        tri_f = const.tile([128, 128], F32, name="tri_f")
        nc.gpsimd.memset(tri_f[:], 1.0)
        nc.gpsimd.affine_select(
            out=tri_f[:], in_=tri_f[:], compare_op=mybir.AluOpType.is_ge,
            fill=0.0, base=0, pattern=[[1, 128]], channel_multiplier=-1)
        tri2 = const.tile([128, 2, 128], BF16, name="tri2")
        nc.vector.tensor_copy(tri2[:, 0], tri_f[:])
        nc.vector.tensor_copy(tri2[:, 1], tri_f[:])
        ones16_f = const.tile([128, 16], F32, name="ones16_f")
        nc.gpsimd.memset(ones16_f[:], 1.0)

        obuf = dram.tile([T, C], F32, name="obuf")
        orec = dram.tile([T // 2, C], F32, name="orec")

        persist = top.enter_context(tc.tile_pool(name="persist", bufs=1))
        wqT = persist.tile([128, CCH, S], BF16, name="wqT")
        wkT = persist.tile([128, CCH, S], BF16, name="wkT")
        wvT = persist.tile([128, CCH, S], BF16, name="wvT")
        woT = persist.tile([128, 4, C], BF16, name="woT")
        kT = persist.tile([128, 4, T], BF16, name="kT")
        vt = persist.tile([128, TT, NH, HD + 1], BF16, name="vt")

        with ExitStack() as body:
            ps_pool = body.enter_context(
                tc.tile_pool(name="ps_pool", bufs=2, space="PSUM"))
            ps_av = body.enter_context(
                tc.tile_pool(name="ps_av", bufs=2, space="PSUM"))
            xtn_pool = body.enter_context(tc.tile_pool(name="xtn", bufs=1))
            qt_pool = body.enter_context(tc.tile_pool(name="qt_pool", bufs=2))
            avt_pool = body.enter_context(
                tc.tile_pool(name="avt_pool", bufs=2))
            p_pool = body.enter_context(tc.tile_pool(name="p_pool", bufs=4))
            rlb_pool = body.enter_context(
                tc.tile_pool(name="rlb_pool", bufs=1))
            o_pool = body.enter_context(tc.tile_pool(name="o_pool", bufs=2))

            xT_r = xT_d.rearrange("(c p) t -> p c t", p=128)

            def proj_start(n):
                xtn = xtn_pool.tile([128, CCH, 512], BF16, name="xtn",
                                    tag="xtn")
                nc.sync.dma_start(xtn[:],
                                  xT_r[:, :, n * 512:(n + 1) * 512])
                if n == 0:
                    wq_r = wqT_d.rearrange("(c p) s -> p c s", p=128)
                    wk_r = wkT_d.rearrange("(c p) s -> p c s", p=128)
                    wv_r = wvT_d.rearrange("(c p) s -> p c s", p=128)
                    for m in range(4):
                        nc.scalar.dma_start(
                            wqT[:, :, m * 128:(m + 1) * 128],
                            wq_r[:, :, m * 128:(m + 1) * 128])
                    for m in range(4):
                        nc.scalar.dma_start(
                            wkT[:, :, m * 128:(m + 1) * 128],
                            wk_r[:, :, m * 128:(m + 1) * 128])
                    nc.scalar.dma_start(wvT[:], wv_r)
                qTc = qt_pool.tile([128, 4, 512], BF16, name="qTc", tag="qTc")
                return xtn, qTc

            def _proj_qk_group(n, xtn, qTc, wT, dst, mp):
                ps = ps_pool.tile([128, 1024], F32, name="ps", tag="ps")
                for half in range(2):
                    m = 2 * mp + half
                    for c in range(CCH):
                        nc.tensor.matmul(
                            ps[:, half * 512:(half + 1) * 512],
                            wT[:, c, m * 128:(m + 1) * 128],
                            xtn[:, c, :],
                            start=(c == 0), stop=(c == CCH - 1))
                if dst is qTc:
                    nc.vector.tensor_copy(
                        qTc[:, 2 * mp:2 * mp + 2, :],
                        ps[:].rearrange("p (a q) -> p a q", a=2))
                else:
                    nc.vector.tensor_copy(
                        dst[:, 2 * mp:2 * mp + 2, n * 512:(n + 1) * 512],
                        ps[:].rearrange("p (a q) -> p a q", a=2))

            def _proj_v_group(n, xtn, tp, ones):
                ps = ps_pool.tile([128, 1024], F32, name="ps", tag="ps")
                for half in range(2):
                    tl = 2 * tp + half
                    for c in range(CCH):
                        nc.tensor.matmul(
                            ps[:, half * 512:(half + 1) * 512],
                            xtn[:, c, tl * 128:(tl + 1) * 128],
                            wvT[:, c, :],
                            start=(c == 0), stop=(c == CCH - 1))
                t0 = 4 * n + 2 * tp
                nc.vector.tensor_copy(
                    vt[:, t0:t0 + 2, :, 0:64],
                    ps[:].rearrange("p (a h d) -> p a h d", a=2, h=NH))
                if ones:
                    nc.scalar.copy(
                        vt[:, t0:t0 + 2, :, 64],
                        ones16_f[:].rearrange("p (a h) -> p a h", a=2))

            def proj_groups(n, xtn, qTc, ones=False, qonly=False):
                gs = []
                wds = ((wqT, qTc),) if qonly else ((wqT, qTc), (wkT, kT))
                for wT, dst in wds:
                    for mp in range(2):
                        gs.append(lambda n=n, xtn=xtn, qTc=qTc, wT=wT,
                                  dst=dst, mp=mp:
                                  _proj_qk_group(n, xtn, qTc, wT, dst, mp))
                if not qonly:
                    for tp in range(2):
                        gs.append(lambda n=n, xtn=xtn, tp=tp, ones=ones:
                                  _proj_v_group(n, xtn, tp, ones))
                return gs

            def proj_kv_groups(n, xtn, qTc, ones=False):
                gs = []
                for mp in range(2):
                    gs.append(lambda n=n, xtn=xtn, qTc=qTc, mp=mp:
                              _proj_qk_group(n, xtn, qTc, wkT, kT, mp))
                for tp in range(2):
                    gs.append(lambda n=n, xtn=xtn, tp=tp, ones=ones:
                              _proj_v_group(n, xtn, tp, ones))
                return gs

            def proj_chunk(n, ones=False):
                xtn, qTc = proj_start(n)
                for g in proj_groups(n, xtn, qTc, ones=ones):
                    g()
                return qTc

            def attention_chunk(i, qTc, fillers=()):
                nk = 4 * i + 4  # k-tiles 0..nk-1
                fillers = list(fillers)
                avc = avt_pool.tile([128, 4, 512], BF16, name="avc",
                                    tag="avc")
                for m in range(4):  # head pairs
                    avp = ps_av.tile([128, 1024], F32, name="avp", tag="avp")
                    av_ps = [avp[:, 0:512], avp[:, 512:1024]]
                    for j in range(nk):
                        r = j - 4 * i
                        lo = max(r, 0) * 128
                        qlo = lo
                        sps = ps_pool.tile([128, 1024], F32, name="sps",
                                           tag="ps")
                        for s2 in range(2):
                            nc.tensor.matmul(
                                sps[:, s2 * 512 + qlo:(s2 + 1) * 512],
                                kT[64 * s2:64 * s2 + 64, m,
                                   j * 128:(j + 1) * 128],
                                qTc[64 * s2:64 * s2 + 64, m, qlo:512],
                                start=True, stop=True)
                        pp = p_pool.tile([128, 1024], BF16, name="pp",
                                         tag="pp")
                        nc.scalar.activation(
                            pp[:].rearrange("p (s q) -> p s q", s=2)
                                [:, :, lo:512],
                            sps[:].rearrange("p (s q) -> p s q", s=2)
                                [:, :, lo:512],
                            EXP, scale=0.125)
                        if r >= 0:
                            blk = pp[:].rearrange(
                                "p (s q) -> p s q", s=2)[:, :, lo:lo + 128]
                            nc.vector.tensor_tensor(blk, blk, tri2[:],
                                                    op=MULT)
                        for s2 in range(2):
                            h = 2 * m + s2
                            nc.tensor.matmul(
                                av_ps[s2][0:65, lo:512],
                                vt[:, j, h, :],
                                pp[:, s2 * 512 + lo:(s2 + 1) * 512],
                                start=(j == 0), stop=(j == nk - 1))
                    # both heads' denominators sit in row 64 of the
                    # combined av tile: one wide reciprocal + one broadcast
                    rlb = rlb_pool.tile([64, 1024], F32, name="rlb",
                                        tag="rlb")
                    nc.vector.reciprocal(rlb[0:1, :], avp[64:65, :])
                    nc.gpsimd.partition_broadcast(rlb[:], rlb[0:1, :])
                    for s2 in range(2):
                        nc.vector.tensor_tensor(
                            avc[64 * s2:64 * s2 + 64, m, :],
                            av_ps[s2][0:64, :],
                            rlb[:, s2 * 512:(s2 + 1) * 512], op=MULT)
                for g in fillers:
                    g()
                return avc

            def oproj_chunk(i, avc):
                for tl in range(4):
                    t = 4 * i + tl
                    pso = ps_av.tile([128, 1024], F32, name="pso",
                                     tag="avp")
                    for nh2 in range(2):
                        for m in range(4):
                            nc.tensor.matmul(
                                pso[:, nh2 * 512:(nh2 + 1) * 512],
                                avc[:, m, tl * 128:(tl + 1) * 128],
                                woT[:, m, nh2 * 512:(nh2 + 1) * 512],
                                start=(m == 0), stop=(m == 3))
                    osb = o_pool.tile([128, C], F32, name="osb", tag="osb")
                    nc.vector.tensor_copy(osb[:], pso[:])
                    nc.sync.dma_start(obuf[t * 128:(t + 1) * 128, :], osb[:])

            def reduce_chunk(i):
                if not collective:
                    nc.sync.dma_start(oh_d[256 * i:256 * (i + 1), :],
                                      obuf[256 * i:256 * (i + 1), :])
                    return
                nc.gpsimd.collective_compute(
                    "ReduceScatter", mybir.AluOpType.add,
                    replica_groups=RG,
                    ins=[obuf[512 * i:512 * (i + 1), :]],
                    outs=[orec[256 * i:256 * (i + 1), :]])
                nc.sync.dma_start(oh_d[256 * i:256 * (i + 1), :],
                                  orec[256 * i:256 * (i + 1), :])

            q0 = x0 = None
            for _it in range(repeat):
                first = _it == 0
                if first:
                    q0 = proj_chunk(0, ones=True)
                    nc.scalar.dma_start(
                        woT[:], woT_d.rearrange("(m p) c -> p m c", p=128))
                else:
                    # q0 was projected during the previous iteration's
                    # att3; k/v of chunk 0 still pending (WAR on kT/vt)
                    for g in proj_kv_groups(0, x0, q0):
                        g()
                x1, q1 = proj_start(1)
                av0 = attention_chunk(0, q0, proj_groups(1, x1, q1,
                                                          ones=first))
                x2, q2 = proj_start(2)
                av1 = attention_chunk(1, q1, proj_groups(2, x2, q2,
                                                          ones=first))
                oproj_chunk(0, av0)
                reduce_chunk(0)
                x3, q3 = proj_start(3)
                av2 = attention_chunk(2, q2, proj_groups(3, x3, q3,
                                                          ones=first))
                oproj_chunk(1, av1)
                reduce_chunk(1)
                if _it < repeat - 1:
                    x0, q0 = proj_start(0)
                    av3 = attention_chunk(3, q3,
                                          proj_groups(0, x0, q0, qonly=True))
                else:
                    av3 = attention_chunk(3, q3)
                oproj_chunk(2, av2)
                reduce_chunk(2)
                oproj_chunk(3, av3)
                reduce_chunk(3)

    nc.compile()
    return nc


def _get_nc():
    if "nc" not in _cache:
        _cache["nc"] = _build_kernel()
    return _cache["nc"]


def _in_maps(x, W_q, W_k, W_v, W_o):
    bf16 = ml_dtypes.bfloat16
    x = np.asarray(x, dtype=bf16)
    W_q = np.asarray(W_q, dtype=bf16)
    W_k = np.asarray(W_k, dtype=bf16)
    W_v = np.asarray(W_v, dtype=bf16)
    W_o = np.asarray(W_o, dtype=bf16)
    maps = []
    for core in range(NCORES):
        b, g = core // 2, core % 2
        sl = slice(g * S, (g + 1) * S)
        maps.append({
            "xT": np.ascontiguousarray(x[b].T),
            "wqT": np.ascontiguousarray(W_q[sl].T),
            "wkT": np.ascontiguousarray(W_k[sl].T),
            "wvT": np.ascontiguousarray(W_v[sl].T),
            "woT": np.ascontiguousarray(W_o[:, sl].T),
        })
    return maps


def _assemble(results):
    out = np.empty((B, T, C), np.float32)
    for b in range(B):
        ev = results[2 * b]["o_half"]
        od = results[2 * b + 1]["o_half"]
        for i in range(QC):
            out[b, 512 * i:512 * i + 256] = ev[256 * i:256 * i + 256]
            out[b, 512 * i + 256:512 * (i + 1)] = od[256 * i:256 * i + 256]
    return out


def kernel(x, W_q, W_k, W_v, W_o):
    nc = _get_nc()
    res = bass_utils.run_bass_kernel_spmd(
        nc, _in_maps(x, W_q, W_k, W_v, W_o), core_ids=list(range(NCORES)))
    return _assemble(res.results)

